# revision 2
# baseline (speedup 1.0000x reference)
"""CrossAttention kernel v4: v3 + fp8e4m3 DoubleRow for q and out-proj.

Wq/Wout/xb/oc are cast to fp8 (weights pre-scaled x16 on host to stay in
e4m3 normal range; the x16 is divided out of kT host-side for q, and out
of the final output host-side for the out projection, with the residual
pre-scaled x16 to match). DoubleRow contracts 256 per MM, halving the
q/out-proj matmul count (16 -> 8 each per chunk).
"""

import numpy as np
import ml_dtypes

import concourse.bass as bass
import concourse.mybir as mybir
import concourse.tile as tile
from concourse import bacc
from concourse.bass_utils import run_bass_kernel_spmd

HEADS = 8
DIM_HEAD = 64
SCALE = DIM_HEAD ** -0.5
DIM = 512
CTX_DIM = 768
N_CTX = 256
HW = 4096
CH = 512
NCHUNK = HW // CH  # 8
B = 8

F32 = mybir.dt.float32
BF16 = mybir.dt.bfloat16
F8 = mybir.dt.float8e4
WSCALE = 16.0  # host pre-scale on Wq/Wout so fp8e4m3 stays in normal range


def build_bass(loop_n=1):
    nc = bacc.Bacc(
        "TRN2",
        target_bir_lowering=False,
        debug=False,
        num_devices=B,
    )

    xb_d = nc.declare_dram_parameter("xb", [128, NCHUNK, 4, CH], F8, isOutput=False)
    xr_d = nc.declare_dram_parameter("xr", [128, NCHUNK, 4, CH], BF16, isOutput=False)
    ctx_d = nc.declare_dram_parameter("ctxT", [128, 6, N_CTX], BF16, isOutput=False)
    wq_d = nc.declare_dram_parameter("wqT", [128, 4, DIM], F8, isOutput=False)
    wk_d = nc.declare_dram_parameter("wkT", [128, 6, DIM], BF16, isOutput=False)
    wv_d = nc.declare_dram_parameter("wvT", [128, 6, DIM], BF16, isOutput=False)
    wo_d = nc.declare_dram_parameter("woutT", [128, 4, DIM], F8, isOutput=False)
    out_d = nc.declare_dram_parameter("out", [128, NCHUNK, 4, CH], BF16, isOutput=True)

    with tile.TileContext(nc) as tc:
        with (
            tc.tile_pool(name="wts", bufs=1) as wts,
            tc.tile_pool(name="kv", bufs=1) as kvp,
            tc.tile_pool(name="xp", bufs=3) as xp,
            tc.tile_pool(name="rxp", bufs=3) as rxp,
            tc.tile_pool(name="qp", bufs=2) as qp,
            tc.tile_pool(name="ep", bufs=10) as ep,
            tc.tile_pool(name="rp", bufs=3) as rp,
            tc.tile_pool(name="ocp", bufs=2) as ocp,
            tc.tile_pool(name="outp", bufs=3) as outp,
            tc.tile_pool(name="psim", bufs=5, space="PSUM") as psim,
            tc.tile_pool(name="pattn", bufs=1, space="PSUM") as pattn,
            tc.tile_pool(name="pq", bufs=1, space="PSUM") as pq,
        ):
            wq_sb = wts.tile([128, 4, DIM], F8)
            nc.sync.dma_start(out=wq_sb, in_=wq_d[:])
            wk_sb = wts.tile([128, 6, DIM], BF16)
            nc.sync.dma_start(out=wk_sb, in_=wk_d[:])
            wv_sb = wts.tile([128, 6, DIM], BF16)
            nc.sync.dma_start(out=wv_sb, in_=wv_d[:])
            wo_sb = wts.tile([128, 4, DIM], F8)
            nc.sync.dma_start(out=wo_sb, in_=wo_d[:])
            ctx_sb = wts.tile([128, 6, N_CTX], BF16)
            nc.sync.dma_start(out=ctx_sb, in_=ctx_d[:])
            ones_sb = wts.tile([128, DIM_HEAD], BF16)
            nc.vector.memset(ones_sb, 1.0)

            for _it in range(loop_n):
                # ---- kT / v ----
                kT_sb = kvp.tile([128, 4, N_CTX], BF16, tag="kT")
                for m in range(4):
                    pt = pq.tile([128, CH], F32, tag="pq")
                    for k in range(6):
                        nc.tensor.matmul(
                            pt[:, :N_CTX],
                            wk_sb[:, k, bass.ts(m, 128)],
                            ctx_sb[:, k, :],
                            start=(k == 0),
                            stop=(k == 5),
                        )
                    nc.scalar.copy(out=kT_sb[:, m, :], in_=pt[:, :N_CTX])

                v_sb = kvp.tile([128, 2, DIM], BF16, tag="v")
                for m in range(2):
                    pt = pq.tile([128, CH], F32, tag="pq")
                    for k in range(6):
                        nc.tensor.matmul(
                            pt,
                            ctx_sb[:, k, bass.ts(m, 128)],
                            wv_sb[:, k, :],
                            start=(k == 0),
                            stop=(k == 5),
                        )
                    nc.scalar.copy(out=v_sb[:, m, :], in_=pt)

                def emit_q_m(q_t, xb_t, m):
                    pt = pq.tile([128, CH], F32, tag="pq")
                    for g in range(2):  # DoubleRow: contracts 2x128 per MM
                        nc.tensor.matmul(
                            pt,
                            wq_sb[:, 2 * g:2 * g + 2, bass.ts(m, 128)],
                            xb_t[:, 2 * g:2 * g + 2, :],
                            start=(g == 0),
                            stop=(g == 1),
                            perf_mode=mybir.MatmulPerfMode.DoubleRow,
                        )
                    nc.vector.tensor_copy(out=q_t[:, m, :], in_=pt)

                def emit_sim(q_t, p):
                    es = []
                    for j in range(2):
                        tiles_j = []
                        for hh in range(2):
                            h0 = hh * 64
                            pt = psim.tile([128, CH], F32, tag="sim")
                            nc.tensor.matmul(
                                pt,
                                kT_sb[h0:h0 + 64, p, bass.ts(j, 128)],
                                q_t[h0:h0 + 64, p, :],
                                start=True,
                                stop=True,
                            )
                            tiles_j.append(pt)
                        for hh in range(2):
                            e_sb = ep.tile([128, CH], BF16, tag="e")
                            nc.scalar.activation(
                                out=e_sb,
                                in_=tiles_j[hh],
                                func=mybir.ActivationFunctionType.Exp,
                            )
                            es.append(e_sb)
                    return es

                def emit_attn(oc_t, p, es):
                    pav = pattn.tile([128, CH], F32, tag="pav")
                    pS = pattn.tile([128, CH], F32, tag="pS")
                    for kj in range(2):
                        for hh in range(2):
                            h = 2 * p + hh
                            h0 = hh * 64
                            nc.tensor.matmul(
                                pav[h0:h0 + 64, :],
                                v_sb[:, kj, bass.ds(h * 64, 64)],
                                es[kj * 2 + hh],
                                start=(kj == 0),
                                stop=(kj == 1),
                                skip_group_check=True,
                            )
                    for kj in range(2):
                        for hh in range(2):
                            h0 = hh * 64
                            nc.tensor.matmul(
                                pS[h0:h0 + 64, :],
                                ones_sb,
                                es[kj * 2 + hh],
                                start=(kj == 0),
                                stop=(kj == 1),
                                skip_group_check=True,
                            )
                    r_sb = rp.tile([128, CH], F32, tag="r")
                    nc.vector.reciprocal_approx_fast(out=r_sb, in_=pS)
                    nc.vector.tensor_mul(out=oc_t[:, p, :], in0=pav, in1=r_sb)

                # prologue: chunk 0 inputs + q(0)
                xb_cur = xp.tile([128, 4, CH], F8, tag="xb")
                nc.sync.dma_start(out=xb_cur, in_=xb_d[:, 0])
                xr_cur = rxp.tile([128, 4, CH], BF16, tag="xr")
                nc.sync.dma_start(out=xr_cur, in_=xr_d[:, 0])
                q_cur = qp.tile([128, 4, CH], BF16, tag="q")
                for m in range(4):
                    emit_q_m(q_cur, xb_cur, m)

                for c in range(NCHUNK):
                    # next-chunk prefetch + q fillers
                    if c + 1 < NCHUNK:
                        xb_nxt = xp.tile([128, 4, CH], F8, tag="xb")
                        nc.sync.dma_start(out=xb_nxt, in_=xb_d[:, c + 1])
                        xr_nxt = rxp.tile([128, 4, CH], BF16, tag="xr")
                        nc.sync.dma_start(out=xr_nxt, in_=xr_d[:, c + 1])
                        q_nxt = qp.tile([128, 4, CH], BF16, tag="q")
                        fillers = [
                            (lambda m=m: emit_q_m(q_nxt, xb_nxt, m))
                            for m in range(4)
                        ]
                    else:
                        xb_nxt = xr_nxt = q_nxt = None
                        fillers = [lambda: None] * 4

                    oc_sb = ocp.tile([128, 4, CH], F8)
                    es0 = emit_sim(q_cur, 0)
                    fillers[0]()
                    es1 = emit_sim(q_cur, 1)
                    emit_attn(oc_sb, 0, es0)
                    fillers[1]()
                    es2 = emit_sim(q_cur, 2)
                    emit_attn(oc_sb, 1, es1)
                    fillers[2]()
                    es3 = emit_sim(q_cur, 3)
                    emit_attn(oc_sb, 2, es2)
                    fillers[3]()
                    emit_attn(oc_sb, 3, es3)

                    o_sb = outp.tile([128, 4, CH], BF16)
                    for m in range(4):
                        pt = pq.tile([128, CH], F32, tag="pq")
                        for g in range(2):
                            nc.tensor.matmul(
                                pt,
                                wo_sb[:, 2 * g:2 * g + 2, bass.ts(m, 128)],
                                oc_sb[:, 2 * g:2 * g + 2, :],
                                start=(g == 0),
                                stop=(g == 1),
                                perf_mode=mybir.MatmulPerfMode.DoubleRow,
                            )
                        nc.vector.tensor_add(
                            out=o_sb[:, m, :],
                            in0=pt,
                            in1=xr_cur[:, m, :],
                        )
                    nc.gpsimd.dma_start(out=out_d[:, c], in_=o_sb)

                    xb_cur, xr_cur, q_cur = xb_nxt, xr_nxt, q_nxt

    nc.compile()
    return nc


_NC_CACHE = None


def _get_nc():
    global _NC_CACHE
    if _NC_CACHE is None:
        _NC_CACHE = build_bass()
    return _NC_CACHE


def _shuffle_pcti(a_f32):
    return np.ascontiguousarray(
        a_f32.reshape(4, 128, NCHUNK, CH).transpose(1, 2, 0, 3)
    )


def make_in_maps(x, context, Wq, Wkv, Wout, bout):
    f = np.float32
    bf = ml_dtypes.bfloat16
    f8 = ml_dtypes.float8_e4m3

    def pm(wT, t, dt=bf):
        return np.ascontiguousarray(
            wT.reshape(t, 128, wT.shape[1]).transpose(1, 0, 2)
        ).astype(dt)

    # q = (16 Wq) x in fp8; the 1/16 is folded into kT's scale.
    # out-proj = (16 Wout) oc in fp8; residual pre-scaled x16 and the
    # whole output divided by 16 host-side in postprocess().
    wqT = pm(np.ascontiguousarray(Wq.T) * np.float32(WSCALE), 4, f8)
    wkT = pm(np.ascontiguousarray(Wkv[:512].T * np.float32(SCALE / WSCALE)), 6)
    wvT = pm(np.ascontiguousarray(Wkv[512:].T), 6)
    woutT = pm(np.ascontiguousarray(Wout.T) * np.float32(WSCALE), 4, f8)
    bout = np.asarray(bout, dtype=f)
    in_maps = []
    for b in range(B):
        xf = np.ascontiguousarray(x[b].reshape(DIM, HW), dtype=f)
        in_maps.append({
            "xb": _shuffle_pcti(xf).astype(f8),
            "xr": _shuffle_pcti((xf + bout[:, None]) * np.float32(WSCALE)).astype(bf),
            "ctxT": pm(np.ascontiguousarray(context[b].T), 6),
            "wqT": wqT,
            "wkT": wkT,
            "wvT": wvT,
            "woutT": woutT,
        })
    return in_maps


def postprocess(raw):
    return (
        raw.transpose(2, 0, 1, 3).reshape(DIM, 64, 64).astype(np.float32)
        * np.float32(1.0 / WSCALE)
    )


def kernel(x, context, Wq, Wkv, Wout, bout):
    x = np.asarray(x)
    context = np.asarray(context)
    nc = _get_nc()
    in_maps = make_in_maps(x, context, np.asarray(Wq), np.asarray(Wkv),
                           np.asarray(Wout), np.asarray(bout))
    res = run_bass_kernel_spmd(nc, in_maps, core_ids=list(range(B)))
    return np.stack([postprocess(res.results[b]["out"]) for b in range(B)], axis=0)


# revision 3
# speedup vs baseline: 1.0580x; 1.0580x over previous
"""CrossAttention kernel v4: v3 + fp8e4m3 DoubleRow for q and out-proj.

Wq/Wout/xb/oc are cast to fp8 (weights pre-scaled x16 on host to stay in
e4m3 normal range; the x16 is divided out of kT host-side for q, and out
of the final output host-side for the out projection, with the residual
pre-scaled x16 to match). DoubleRow contracts 256 per MM, halving the
q/out-proj matmul count (16 -> 8 each per chunk).
"""

import numpy as np
import ml_dtypes

import concourse.bass as bass
import concourse.mybir as mybir
import concourse.tile as tile
from concourse import bacc
from concourse.bass_utils import run_bass_kernel_spmd

HEADS = 8
DIM_HEAD = 64
SCALE = DIM_HEAD ** -0.5
DIM = 512
CTX_DIM = 768
N_CTX = 256
HW = 4096
CH = 512
NCHUNK = HW // CH  # 8
B = 8

F32 = mybir.dt.float32
BF16 = mybir.dt.bfloat16
F8 = mybir.dt.float8e4
WSCALE = 16.0  # host pre-scale on Wq/Wout so fp8e4m3 stays in normal range


def build_bass(loop_n=1):
    nc = bacc.Bacc(
        "TRN2",
        target_bir_lowering=False,
        debug=False,
        num_devices=B,
    )

    xb_d = nc.declare_dram_parameter("xb", [128, NCHUNK, 4, CH], F8, isOutput=False)
    xr_d = nc.declare_dram_parameter("xr", [128, NCHUNK, 4, CH], BF16, isOutput=False)
    ctx_d = nc.declare_dram_parameter("ctxT", [128, 6, N_CTX], BF16, isOutput=False)
    wq_d = nc.declare_dram_parameter("wqT", [128, 4, DIM], F8, isOutput=False)
    wk_d = nc.declare_dram_parameter("wkT", [128, 6, DIM], BF16, isOutput=False)
    wv_d = nc.declare_dram_parameter("wvT", [128, 6, DIM], BF16, isOutput=False)
    wo_d = nc.declare_dram_parameter("woutT", [128, 4, DIM], F8, isOutput=False)
    out_d = nc.declare_dram_parameter("out", [128, NCHUNK, 4, CH], BF16, isOutput=True)

    with tile.TileContext(nc) as tc:
        with (
            tc.tile_pool(name="wts", bufs=1) as wts,
            tc.tile_pool(name="kv", bufs=1) as kvp,
            tc.tile_pool(name="xp", bufs=3) as xp,
            tc.tile_pool(name="rxp", bufs=3) as rxp,
            tc.tile_pool(name="qp", bufs=2) as qp,
            tc.tile_pool(name="ep", bufs=10) as ep,
            tc.tile_pool(name="rp", bufs=3) as rp,
            tc.tile_pool(name="ocp", bufs=2) as ocp,
            tc.tile_pool(name="outp", bufs=3) as outp,
            tc.tile_pool(name="psim", bufs=5, space="PSUM") as psim,
            tc.tile_pool(name="pattn", bufs=1, space="PSUM") as pattn,
            tc.tile_pool(name="pq", bufs=1, space="PSUM") as pq,
        ):
            wq_sb = wts.tile([128, 4, DIM], F8)
            nc.sync.dma_start(out=wq_sb, in_=wq_d[:])
            wk_sb = wts.tile([128, 6, DIM], BF16)
            nc.sync.dma_start(out=wk_sb, in_=wk_d[:])
            wv_sb = wts.tile([128, 6, DIM], BF16)
            nc.sync.dma_start(out=wv_sb, in_=wv_d[:])
            wo_sb = wts.tile([128, 4, DIM], F8)
            nc.sync.dma_start(out=wo_sb, in_=wo_d[:])
            ctx_sb = wts.tile([128, 6, N_CTX], BF16)
            nc.sync.dma_start(out=ctx_sb, in_=ctx_d[:])
            ones_sb = wts.tile([128, DIM_HEAD], BF16)
            nc.vector.memset(ones_sb, 1.0)

            for _it in range(loop_n):
                # ---- kT / v ----
                kT_sb = kvp.tile([128, 4, N_CTX], BF16, tag="kT")
                for m in range(4):
                    # alternate PSUM banks so m+1's MMs don't WAR-wait
                    # on m's PSUM->SBUF drain
                    if m % 2 == 0:
                        pt = pq.tile([128, CH], F32, tag="pq")
                    else:
                        pt = pattn.tile([128, CH], F32, tag="pav")
                    for k in range(6):
                        nc.tensor.matmul(
                            pt[:, :N_CTX],
                            wk_sb[:, k, bass.ts(m, 128)],
                            ctx_sb[:, k, :],
                            start=(k == 0),
                            stop=(k == 5),
                        )
                    nc.scalar.copy(out=kT_sb[:, m, :], in_=pt[:, :N_CTX])

                v_sb = kvp.tile([128, 2, DIM], BF16, tag="v")
                for m in range(2):
                    if m % 2 == 0:
                        pt = pq.tile([128, CH], F32, tag="pq")
                    else:
                        pt = pattn.tile([128, CH], F32, tag="pS")
                    for k in range(6):
                        nc.tensor.matmul(
                            pt,
                            ctx_sb[:, k, bass.ts(m, 128)],
                            wv_sb[:, k, :],
                            start=(k == 0),
                            stop=(k == 5),
                        )
                    nc.scalar.copy(out=v_sb[:, m, :], in_=pt)

                def emit_q_m(q_t, xb_t, m):
                    pt = pq.tile([128, CH], F32, tag="pq")
                    for g in range(2):  # DoubleRow: contracts 2x128 per MM
                        nc.tensor.matmul(
                            pt,
                            wq_sb[:, 2 * g:2 * g + 2, bass.ts(m, 128)],
                            xb_t[:, 2 * g:2 * g + 2, :],
                            start=(g == 0),
                            stop=(g == 1),
                            perf_mode=mybir.MatmulPerfMode.DoubleRow,
                        )
                    nc.vector.tensor_copy(out=q_t[:, m, :], in_=pt)

                def emit_sim(q_t, p):
                    es = []
                    for j in range(2):
                        tiles_j = []
                        for hh in range(2):
                            h0 = hh * 64
                            pt = psim.tile([128, CH], F32, tag="sim")
                            nc.tensor.matmul(
                                pt,
                                kT_sb[h0:h0 + 64, p, bass.ts(j, 128)],
                                q_t[h0:h0 + 64, p, :],
                                start=True,
                                stop=True,
                            )
                            tiles_j.append(pt)
                        for hh in range(2):
                            e_sb = ep.tile([128, CH], BF16, tag="e")
                            nc.scalar.activation(
                                out=e_sb,
                                in_=tiles_j[hh],
                                func=mybir.ActivationFunctionType.Exp,
                            )
                            es.append(e_sb)
                    return es

                def emit_attn(oc_t, p, es):
                    pav = pattn.tile([128, CH], F32, tag="pav")
                    pS = pattn.tile([128, CH], F32, tag="pS")
                    for kj in range(2):
                        for hh in range(2):
                            h = 2 * p + hh
                            h0 = hh * 64
                            nc.tensor.matmul(
                                pav[h0:h0 + 64, :],
                                v_sb[:, kj, bass.ds(h * 64, 64)],
                                es[kj * 2 + hh],
                                start=(kj == 0),
                                stop=(kj == 1),
                                skip_group_check=True,
                            )
                    for kj in range(2):
                        for hh in range(2):
                            h0 = hh * 64
                            nc.tensor.matmul(
                                pS[h0:h0 + 64, :],
                                ones_sb,
                                es[kj * 2 + hh],
                                start=(kj == 0),
                                stop=(kj == 1),
                                skip_group_check=True,
                            )
                    r_sb = rp.tile([128, CH], F32, tag="r")
                    nc.vector.reciprocal_approx_fast(out=r_sb, in_=pS)
                    nc.vector.tensor_mul(out=oc_t[:, p, :], in0=pav, in1=r_sb)

                # prologue: chunk 0 inputs + q(0)
                xb_cur = xp.tile([128, 4, CH], F8, tag="xb")
                nc.sync.dma_start(out=xb_cur, in_=xb_d[:, 0])
                xr_cur = rxp.tile([128, 4, CH], BF16, tag="xr")
                nc.sync.dma_start(out=xr_cur, in_=xr_d[:, 0])
                q_cur = qp.tile([128, 4, CH], BF16, tag="q")
                for m in range(4):
                    emit_q_m(q_cur, xb_cur, m)

                for c in range(NCHUNK):
                    # next-chunk prefetch + q fillers
                    if c + 1 < NCHUNK:
                        xb_nxt = xp.tile([128, 4, CH], F8, tag="xb")
                        nc.sync.dma_start(out=xb_nxt, in_=xb_d[:, c + 1])
                        xr_nxt = rxp.tile([128, 4, CH], BF16, tag="xr")
                        nc.sync.dma_start(out=xr_nxt, in_=xr_d[:, c + 1])
                        q_nxt = qp.tile([128, 4, CH], BF16, tag="q")
                        fillers = [
                            (lambda m=m: emit_q_m(q_nxt, xb_nxt, m))
                            for m in range(4)
                        ]
                    else:
                        xb_nxt = xr_nxt = q_nxt = None
                        fillers = [lambda: None] * 4

                    oc_sb = ocp.tile([128, 4, CH], F8)
                    es0 = emit_sim(q_cur, 0)
                    fillers[0]()
                    es1 = emit_sim(q_cur, 1)
                    emit_attn(oc_sb, 0, es0)
                    fillers[1]()
                    es2 = emit_sim(q_cur, 2)
                    emit_attn(oc_sb, 1, es1)
                    fillers[2]()
                    es3 = emit_sim(q_cur, 3)
                    emit_attn(oc_sb, 2, es2)
                    fillers[3]()
                    emit_attn(oc_sb, 3, es3)

                    o_sb = outp.tile([128, 4, CH], BF16)
                    for m in range(4):
                        if m % 2 == 0:
                            pt = pq.tile([128, CH], F32, tag="pq")
                        else:
                            pt = pattn.tile([128, CH], F32, tag="pav")
                        for g in range(2):
                            nc.tensor.matmul(
                                pt,
                                wo_sb[:, 2 * g:2 * g + 2, bass.ts(m, 128)],
                                oc_sb[:, 2 * g:2 * g + 2, :],
                                start=(g == 0),
                                stop=(g == 1),
                                perf_mode=mybir.MatmulPerfMode.DoubleRow,
                            )
                        nc.vector.tensor_add(
                            out=o_sb[:, m, :],
                            in0=pt,
                            in1=xr_cur[:, m, :],
                        )
                    nc.gpsimd.dma_start(out=out_d[:, c], in_=o_sb)

                    xb_cur, xr_cur, q_cur = xb_nxt, xr_nxt, q_nxt

    nc.compile()
    return nc


_NC_CACHE = None


def _get_nc():
    global _NC_CACHE
    if _NC_CACHE is None:
        _NC_CACHE = build_bass()
    return _NC_CACHE


def _shuffle_pcti(a_f32):
    return np.ascontiguousarray(
        a_f32.reshape(4, 128, NCHUNK, CH).transpose(1, 2, 0, 3)
    )


def make_in_maps(x, context, Wq, Wkv, Wout, bout):
    f = np.float32
    bf = ml_dtypes.bfloat16
    f8 = ml_dtypes.float8_e4m3

    def pm(wT, t, dt=bf):
        return np.ascontiguousarray(
            wT.reshape(t, 128, wT.shape[1]).transpose(1, 0, 2)
        ).astype(dt)

    # q = (16 Wq) x in fp8; the 1/16 is folded into kT's scale.
    # out-proj = (16 Wout) oc in fp8; residual pre-scaled x16 and the
    # whole output divided by 16 host-side in postprocess().
    wqT = pm(np.ascontiguousarray(Wq.T) * np.float32(WSCALE), 4, f8)
    wkT = pm(np.ascontiguousarray(Wkv[:512].T * np.float32(SCALE / WSCALE)), 6)
    wvT = pm(np.ascontiguousarray(Wkv[512:].T), 6)
    woutT = pm(np.ascontiguousarray(Wout.T) * np.float32(WSCALE), 4, f8)
    bout = np.asarray(bout, dtype=f)
    in_maps = []
    for b in range(B):
        xf = np.ascontiguousarray(x[b].reshape(DIM, HW), dtype=f)
        in_maps.append({
            "xb": _shuffle_pcti(xf).astype(f8),
            "xr": _shuffle_pcti((xf + bout[:, None]) * np.float32(WSCALE)).astype(bf),
            "ctxT": pm(np.ascontiguousarray(context[b].T), 6),
            "wqT": wqT,
            "wkT": wkT,
            "wvT": wvT,
            "woutT": woutT,
        })
    return in_maps


def postprocess(raw):
    return (
        raw.transpose(2, 0, 1, 3).reshape(DIM, 64, 64).astype(np.float32)
        * np.float32(1.0 / WSCALE)
    )


def kernel(x, context, Wq, Wkv, Wout, bout):
    x = np.asarray(x)
    context = np.asarray(context)
    nc = _get_nc()
    in_maps = make_in_maps(x, context, np.asarray(Wq), np.asarray(Wkv),
                           np.asarray(Wout), np.asarray(bout))
    res = run_bass_kernel_spmd(nc, in_maps, core_ids=list(range(B)))
    return np.stack([postprocess(res.results[b]["out"]) for b in range(B)], axis=0)
